# revision 2
# baseline (speedup 1.0000x reference)
"""Trainium2 Bass kernel for Derivative1D: y[:, i, :] = x[:, i+1, :] - x[:, i, :].

Full input x: [64, 16384, 32] f32; full output y: [64, 16383, 32] f32.
Sharding: pure data parallel over batch — 8 batches per core on 8 cores.

Per core, each batch's (L, C) block is a contiguous stream of L*C = 524288
f32; the stencil in flat space is y_flat[j] = x_flat[j+32] - x_flat[j]
(shift by exactly C=32 elements).  Each batch is loaded once as a
[128 x (4096+32)] tile with a 32-element overlapping window per partition
(single DMA), one DVE subtract produces all 4096 outputs per partition,
and results stream back out.  The last batch of the shard can't use the
overlapping window for its final partition (would read past the end of
the input tensor), so it takes a separate 32-element halo DMA instead.
"""

import sys

if "/opt/trn_rl_repo" not in sys.path:
    sys.path.insert(0, "/opt/trn_rl_repo")

import numpy as np

import concourse.bass as bass
import concourse.tile as tile
from concourse import bacc, mybir

B, L, C = 64, 16384, 32
NCORES = 8
BS = B // NCORES            # 8 batches per core
NF = L * C                  # 524288 flat input elements per batch
OF = (L - 1) * C            # 524256 flat output elements per batch
P = 128                     # SBUF partitions
F = NF // P                 # 4096 elements per partition per batch
H = C                       # halo: shift distance in flat space


def build_nc(repeat: int = 1, bufs: int = 4):
    """Build the per-core Bass/Tile program (same program on all 8 cores)."""
    nc = bacc.Bacc(
        "TRN2",
        target_bir_lowering=False,
        debug=False,
        num_devices=NCORES,
        enable_partition_id=False,
    )
    x = nc.dram_tensor("x", [BS, L, C], mybir.dt.float32, kind="ExternalInput")
    y = nc.dram_tensor("y", [BS, L - 1, C], mybir.dt.float32, kind="ExternalOutput")

    with tile.TileContext(nc) as tc:
        with (
            tc.tile_pool(name="xin", bufs=bufs) as xin,
            tc.tile_pool(name="yout", bufs=bufs) as yout,
        ):
            for _ in range(repeat):
                for b in range(BS):
                    t = xin.tile([P, F + H], mybir.dt.float32)
                    if b < BS - 1:
                        # One DMA, overlapping window: partition p reads
                        # x_flat[b*NF + p*F : p*F + F + H]; partition 127's
                        # tail spills into batch b+1 (garbage, never stored).
                        nc.sync.dma_start(
                            t[:, :], bass.AP(x, b * NF, [[F, P], [1, F + H]])
                        )
                    else:
                        nc.sync.dma_start(
                            t[:, 0:F], bass.AP(x, b * NF, [[F, P], [1, F]])
                        )
                        nc.sync.dma_start(
                            t[0 : P - 1, F : F + H],
                            bass.AP(x, b * NF + F, [[F, P - 1], [1, H]]),
                        )
                        # Last partition's halo would read past the end of x;
                        # fill it from a harmless in-bounds spot instead (the
                        # lane-127 tail it feeds is never stored).
                        nc.sync.dma_start(
                            t[P - 1 : P, F : F + H],
                            bass.AP(x, 0, [[H, 1], [1, H]]),
                        )

                    o = yout.tile([P, F], mybir.dt.float32)
                    nc.vector.tensor_sub(o[:, 0:F], t[:, H : F + H], t[:, 0:F])

                    # Store on the ACT HWDGE ring so loads (SP ring) and
                    # stores proceed on separate physical descriptor rings.
                    nc.scalar.dma_start(
                        bass.AP(y, b * OF, [[F, P - 1], [1, F]]), o[0 : P - 1, :]
                    )
                    nc.scalar.dma_start(
                        bass.AP(y, b * OF + (P - 1) * F, [[F - H, 1], [1, F - H]]),
                        o[P - 1 : P, 0 : F - H],
                    )

    nc.compile()
    return nc


_NC_CACHE = {}


def _get_nc(repeat: int = 1):
    if repeat not in _NC_CACHE:
        _NC_CACHE[repeat] = build_nc(repeat)
    return _NC_CACHE[repeat]


def kernel(**inputs: np.ndarray) -> np.ndarray:
    x = np.ascontiguousarray(inputs["x"], dtype=np.float32)
    assert x.shape == (B, L, C), x.shape

    from concourse.bass_utils import run_bass_kernel_spmd

    nc = _get_nc()
    in_maps = [
        {"x": np.ascontiguousarray(x[c * BS : (c + 1) * BS])} for c in range(NCORES)
    ]
    res = run_bass_kernel_spmd(nc, in_maps, core_ids=list(range(NCORES)))
    return np.concatenate([r["y"] for r in res.results], axis=0)


# revision 9
# speedup vs baseline: 8.3061x; 8.3061x over previous
"""Trainium2 Bass kernel for Derivative1D: y[:, i, :] = x[:, i+1, :] - x[:, i, :].

Full input x: [64, 16384, 32] f32; full output y: [64, 16383, 32] f32.
Sharding: pure data parallel over batch — 8 batches per core on 8 cores.

Layout (per core): each batch's (L, C) block is a contiguous stream of
L*C = 524288 f32, and the stencil in flat space is
y_flat[j] = x_flat[j+32] - x_flat[j] (shift by exactly C = 32 elements).
Batches are processed in fused groups of 4 because the fused output,
4*(L-1)*C = 2097024 = 128 * 16383, splits perfectly across 128 SBUF
partitions: partition p owns output elements [p*16383, (p+1)*16383) of the
group's output stream, and batch boundaries land exactly at partitions
32/64/96 (524256 = 32*16383).  Partition p = 32*q + i then needs input
x[batch q][i*16383 : i*16383 + 16383 + 32] — the final partition's window
ends exactly at the end of the batch, so the 32-element halo never reads
out of bounds anywhere.

Each group is processed in free-dim chunks: one [128, Fc+32] load (a 3D
access pattern over (batch-in-group, partition-in-batch, element)), one DVE
subtract of the two 32-shifted views, and the store split into four
32-partition DMAs.  Loads use the sync HWDGE ring (HBM->SBUF descriptors
spread over all 16 SDMA engines by the SBUF-port mapping).  Stores use
SWDGE (gpsimd): a single SBUF->HBM dma_start lands on ONE SDMA engine
(~27 GB/s), so the 4-way split plus multi-buffered pipelining keeps ~16
store DMAs in flight and engages all 16 engines.
"""

import sys

if "/opt/trn_rl_repo" not in sys.path:
    sys.path.insert(0, "/opt/trn_rl_repo")

import numpy as np

import concourse.bass as bass
import concourse.tile as tile
from concourse import bacc, mybir

B, L, C = 64, 16384, 32
NCORES = 8
BS = B // NCORES            # 8 batches per core
NF = L * C                  # 524288 flat input elements per batch
OF = (L - 1) * C            # 524256 flat output elements per batch
P = 128                     # SBUF partitions
H = C                       # halo: shift distance in flat space
G = 4                       # batches fused per group
NGROUP = BS // G            # 2 groups per core
FP = OF // 32               # 16383 output elements per partition per group
PB = P // G                 # 32 partitions per batch within a group
NCHUNK = 4                  # free-dim chunks per group
FC = 4096                   # chunk size (last chunk is 4095)
NSLICE = 4                  # 32-partition store slices per chunk


def build_nc(repeat: int = 1, in_bufs: int = 6, out_bufs: int = 6):
    """Build the per-core Bass/Tile program (same program on all 8 cores)."""
    nc = bacc.Bacc(
        "TRN2",
        target_bir_lowering=False,
        debug=False,
        num_devices=NCORES,
        enable_partition_id=False,
    )
    x = nc.dram_tensor("x", [BS, L, C], mybir.dt.float32, kind="ExternalInput")
    y = nc.dram_tensor("y", [BS, L - 1, C], mybir.dt.float32, kind="ExternalOutput")

    with tile.TileContext(nc) as tc:
        with (
            tc.tile_pool(name="xin", bufs=in_bufs) as xin,
            tc.tile_pool(name="yout", bufs=out_bufs) as yout,
        ):
            for _ in range(repeat):
                for g in range(NGROUP):
                    for c in range(NCHUNK):
                        fc = FP - c * FC if c == NCHUNK - 1 else FC  # 4096/4095
                        t = xin.tile([P, FC + H], mybir.dt.float32)
                        # Interleaved partition layout: partition p holds
                        # window pin = p//4 of batch q = p%4.  The outermost
                        # access-pattern dim (32 windows) is what HWDGE
                        # round-robins across SDMA engines, so keep it large.
                        nc.sync.dma_start(
                            t[:, 0 : fc + H],
                            bass.AP(
                                x,
                                g * G * NF + c * FC,
                                [[FP, PB], [NF, G], [1, fc + H]],
                            ),
                        )
                        o = yout.tile([P, FC], mybir.dt.float32)
                        nc.vector.tensor_sub(
                            o[:, 0:fc], t[:, H : fc + H], t[:, 0:fc]
                        )
                        # Store in free-dim quarters: each SWDGE dma_start
                        # lands on one SDMA engine, so 4 slices/chunk plus
                        # pipelining keeps all 16 engines fed.
                        fs = (fc + NSLICE - 1) // NSLICE
                        for j in range(NSLICE):
                            fj = min(fs, fc - j * fs)
                            nc.gpsimd.dma_start(
                                bass.AP(
                                    y,
                                    g * G * OF + c * FC + j * fs,
                                    [[FP, PB], [OF, G], [1, fj]],
                                ),
                                o[:, j * fs : j * fs + fj],
                            )

    nc.compile()
    return nc


_NC_CACHE = {}


def _get_nc(repeat: int = 1):
    if repeat not in _NC_CACHE:
        _NC_CACHE[repeat] = build_nc(repeat)
    return _NC_CACHE[repeat]


def kernel(**inputs: np.ndarray) -> np.ndarray:
    x = np.ascontiguousarray(inputs["x"], dtype=np.float32)
    assert x.shape == (B, L, C), x.shape

    from concourse.bass_utils import run_bass_kernel_spmd

    nc = _get_nc()
    in_maps = [
        {"x": np.ascontiguousarray(x[c * BS : (c + 1) * BS])} for c in range(NCORES)
    ]
    res = run_bass_kernel_spmd(nc, in_maps, core_ids=list(range(NCORES)))
    return np.concatenate([r["y"] for r in res.results], axis=0)


# revision 11
# speedup vs baseline: 8.3140x; 1.0009x over previous
"""Trainium2 Bass kernel for Derivative1D: y[:, i, :] = x[:, i+1, :] - x[:, i, :].

Full input x: [64, 16384, 32] f32; full output y: [64, 16383, 32] f32.
Sharding: pure data parallel over batch — 8 batches per core on 8 cores.

Layout (per core): each batch's (L, C) block is a contiguous stream of
L*C = 524288 f32, and the stencil in flat space is
y_flat[j] = x_flat[j+32] - x_flat[j] (shift by exactly C = 32 elements).
Batches are processed in fused groups of 4 because the fused output,
4*(L-1)*C = 2097024 = 128 * 16383, splits perfectly across 128 SBUF
partitions: partition p owns output elements [p*16383, (p+1)*16383) of the
group's output stream, and batch boundaries land exactly at partitions
32/64/96 (524256 = 32*16383).  Partition p = 32*q + i then needs input
x[batch q][i*16383 : i*16383 + 16383 + 32] — the final partition's window
ends exactly at the end of the batch, so the 32-element halo never reads
out of bounds anywhere.

Each group is processed in free-dim chunks: one [128, Fc+32] load (a 3D
access pattern over (batch-in-group, partition-in-batch, element)), one DVE
subtract of the two 32-shifted views, and the store split into four
32-partition DMAs.  Loads use the sync HWDGE ring (HBM->SBUF descriptors
spread over all 16 SDMA engines by the SBUF-port mapping).  Stores use
SWDGE (gpsimd): a single SBUF->HBM dma_start lands on ONE SDMA engine
(~27 GB/s), so the 4-way split plus multi-buffered pipelining keeps ~16
store DMAs in flight and engages all 16 engines.
"""

import sys

if "/opt/trn_rl_repo" not in sys.path:
    sys.path.insert(0, "/opt/trn_rl_repo")

import numpy as np

import concourse.bass as bass
import concourse.tile as tile
from concourse import bacc, mybir

B, L, C = 64, 16384, 32
NCORES = 8
BS = B // NCORES            # 8 batches per core
NF = L * C                  # 524288 flat input elements per batch
OF = (L - 1) * C            # 524256 flat output elements per batch
P = 128                     # SBUF partitions
H = C                       # halo: shift distance in flat space
G = 4                       # batches fused per group
NGROUP = BS // G            # 2 groups per core
FP = OF // 32               # 16383 output elements per partition per group
PB = P // G                 # 32 partitions per batch within a group
NCHUNK = 4                  # free-dim chunks per group
FC = 4096                   # chunk size (last chunk is 4095)
NSLICE = 4                  # 32-partition store slices per chunk


def build_nc(repeat: int = 1, in_bufs: int = 6, out_bufs: int = 6):
    """Build the per-core Bass/Tile program (same program on all 8 cores)."""
    nc = bacc.Bacc(
        "TRN2",
        target_bir_lowering=False,
        debug=False,
        num_devices=NCORES,
        enable_partition_id=False,
    )
    x = nc.dram_tensor("x", [BS, L, C], mybir.dt.float32, kind="ExternalInput")
    y = nc.dram_tensor("y", [BS, L - 1, C], mybir.dt.float32, kind="ExternalOutput")

    with tile.TileContext(nc) as tc:
        with (
            tc.tile_pool(name="xin", bufs=in_bufs) as xin,
            tc.tile_pool(name="yout", bufs=out_bufs) as yout,
        ):
            for _ in range(repeat):
                for g in range(NGROUP):
                    for c in range(NCHUNK):
                        fc = FP - c * FC if c == NCHUNK - 1 else FC  # 4096/4095
                        t = xin.tile([P, FC + H], mybir.dt.float32)
                        # Interleaved partition layout: partition p holds
                        # window pin = p//4 of batch q = p%4.  The outermost
                        # access-pattern dim (32 windows) is what HWDGE
                        # round-robins across SDMA engines, so keep it large.
                        nc.sync.dma_start(
                            t[:, 0 : fc + H],
                            bass.AP(
                                x,
                                g * G * NF + c * FC,
                                [[FP, PB], [NF, G], [1, fc + H]],
                            ),
                        )
                        o = yout.tile([P, FC], mybir.dt.float32)
                        nc.vector.tensor_sub(
                            o[:, 0:fc], t[:, H : fc + H], t[:, 0:fc]
                        )
                        # Store in free-dim quarters: each SWDGE dma_start
                        # lands on one SDMA engine, so 4 slices/chunk plus
                        # pipelining keeps all 16 engines fed.
                        fs = (fc + NSLICE - 1) // NSLICE
                        for j in range(NSLICE):
                            fj = min(fs, fc - j * fs)
                            nc.gpsimd.dma_start(
                                bass.AP(
                                    y,
                                    g * G * OF + c * FC + j * fs,
                                    [[FP, PB], [OF, G], [1, fj]],
                                ),
                                o[:, j * fs : j * fs + fj],
                            )

    nc.compile()
    return nc


_NC_CACHE = {}


def _get_nc(repeat: int = 1):
    if repeat not in _NC_CACHE:
        _NC_CACHE[repeat] = build_nc(repeat)
    return _NC_CACHE[repeat]


def kernel(**inputs: np.ndarray) -> np.ndarray:
    x = np.ascontiguousarray(inputs["x"], dtype=np.float32)
    assert x.shape == (B, L, C), x.shape

    from concourse.bass_utils import run_bass_kernel_spmd

    nc = _get_nc()
    in_maps = [
        {"x": np.ascontiguousarray(x[c * BS : (c + 1) * BS])} for c in range(NCORES)
    ]
    res = run_bass_kernel_spmd(nc, in_maps, core_ids=list(range(NCORES)))
    return np.concatenate([r["y"] for r in res.results], axis=0)
